# revision 65
# baseline (speedup 1.0000x reference)
"""Trainium2 Bass kernel for the two-tower GCN (nn_GCN2).

Distribution: nodes partitioned by destination range across 8 cores
(graph parallel). All floating-point math runs on device across 3 SPMD
launches; the host only does index manipulation (edge sorting, row
gathering by static indices, dtype casts of inputs) and the inter-launch
reshard/halo-exchange, exactly like the sharding contract allows:

  A: xw  = x @ [W1|W3]                 (node-sharded dense matmul, fp8 out)
  B: h   = relu(spmm(A, xw) + b); hw2 = h @ [W2|W4]    (per dst window)
  C: o   = spmm(A, hw2) + b; gated fusion; log_softmax (per dst window)

The irregular gather of source features is resolved on the host between
launches: since the edge list is static, the per-edge message stream
msgs[chunk, slot, :] = table[src[chunk, slot]] is a pure row-gather of
the previous launch's output, staged partition-major so the device
streams it at full contiguous-DMA bandwidth. The segment-sum runs on the
tensor engine: edges are sorted by destination, so each chunk of 128
edges lands in one 32-wide destination window and
psum[:, win] += msgs_chunk^T @ sel_chunk with a host-built fp8 selector
sel[slot, d] = val * (dst_local == d).
"""
from contextlib import ExitStack

import numpy as np

import concourse.bass as bass
import concourse.tile as tile
from concourse import bacc, mybir
from concourse.bass_utils import run_bass_kernel_spmd
from concourse.masks import make_identity

P = 128
NCORES = 8
N = 50000
E = 800000
NFEAT = 512
NHID = 128
NCLASS = 40
NLOC = N // NCORES             # 6250 real nodes per core
NT128 = 50                     # 128-row blocks per core
NLOCP = NT128 * P              # 6400 padded rows per core (slack for packing)
W = 32                         # dst window width (sel columns)
NWIN = NLOCP // W              # 200 windows per core
SUP = 512                      # dsts per PSUM supertile
NSUP = (NLOCP + SUP - 1) // SUP  # 13 (last one is 256 wide)

f16 = mybir.dt.float16
f32 = mybir.dt.float32
f8 = mybir.dt.float8e4
ACT = mybir.ActivationFunctionType
ALU = mybir.AluOpType
F8NP = mybir.dt.np(f8)


def _cdiv(a, b):
    return (a + b - 1) // b


# ---------------------------------------------------------------- host prep

def balance_rows(degs):
    """Assign each core's nodes to 32-slot windows so that both towers'
    per-window edge counts stay <= 4*128 (pure index manipulation).

    Returns row_of: [N] padded row index of each node within its core.
    """
    deg0, deg1 = degs
    row_of = np.empty(N, np.int64)
    for c in range(NCORES):
        nodes = np.arange(c * NLOC, (c + 1) * NLOC)
        nodes = nodes[np.argsort(-(deg0[nodes] + deg1[nodes]), kind="stable")]
        s0 = np.zeros(NWIN)
        s1 = np.zeros(NWIN)
        cnt = np.zeros(NWIN, np.int64)
        for nd in nodes:
            m = np.where(cnt < 32,
                         np.maximum(s0 + deg0[nd], s1 + deg1[nd]), 1e18)
            w = int(np.argmin(m))
            row_of[nd] = w * W + cnt[w]
            s0[w] += deg0[nd]
            s1[w] += deg1[nd]
            cnt[w] += 1
    return row_of


class TowerPlan:
    """Edge preprocessing for one tower (one graph).

    Sorts each core's in-edges by destination window (32-wide), splits
    them into chunks of 128 slots, pads every (window) to the max chunk
    count over cores so all cores run one program, and records per-slot
    (src, dst_col, val).

    Produces:
      nch          : padded chunk count (same for all cores)
      cs           : [NWIN+1] chunk range per window
      srcs         : [NCORES, nch*128] int32 source row, -1 for pads
      sel          : [NCORES, 128, nch, W] f16 selector (val at dst col)
    """

    def __init__(self, edge_index, edge_vals, row_of):
        src = np.asarray(edge_index[0]).astype(np.int64)
        dst = np.asarray(edge_index[1]).astype(np.int64)
        vals = np.asarray(edge_vals).astype(np.float32)

        core = dst // NLOC
        ldst = row_of[dst]
        win = ldst // W
        col = ldst - win * W

        counts = np.zeros((NCORES, NWIN), np.int64)
        np.add.at(counts, (core, win), 1)
        chunk_cnt = np.maximum(_cdiv(counts, P).max(axis=0), 1)  # [NWIN]
        self.cs = np.concatenate([[0], np.cumsum(chunk_cnt)])
        self.nch = int(self.cs[-1])

        order = np.lexsort((ldst, win, core))
        so_core, so_win = core[order], win[order]
        so_src, so_col, so_val = src[order], col[order], vals[order]
        gkey = so_core * NWIN + so_win
        gstart = np.r_[0, np.flatnonzero(np.diff(gkey)) + 1]
        glen = np.diff(np.r_[gstart, len(gkey)])
        rank = np.arange(len(gkey)) - np.repeat(gstart, glen)
        slot = (self.cs[so_win] * P + rank).astype(np.int64)

        nslot = self.nch * P
        self.srcs = np.full((NCORES, nslot), -1, np.int32)
        cola = np.zeros((NCORES, nslot), np.int64)
        vala = np.zeros((NCORES, nslot), np.float32)
        flat = so_core * nslot + slot
        self.srcs.reshape(-1)[flat] = so_src.astype(np.int32)
        cola.reshape(-1)[flat] = so_col
        vala.reshape(-1)[flat] = so_val

        # compact per-slot forms for on-device sel builds
        self.dl = np.ascontiguousarray(
            cola.reshape(NCORES, self.nch, P).transpose(0, 2, 1)
        ).astype(np.float32)
        self.vl = np.ascontiguousarray(
            vala.reshape(NCORES, self.nch, P).transpose(0, 2, 1)
        ).astype(np.float32)

        # sel[c, p, ch, col] = val of slot (ch, p) if its dst col matches
        sel = np.zeros((NCORES, self.nch, P, W), np.float16)
        ci = np.arange(nslot) // P
        pi = np.arange(nslot) % P
        v8 = vala.astype(np.float16)
        for c in range(NCORES):
            sel[c, ci, pi, cola[c]] = v8[c]
            # pads: col 0 with val 0 already zero
        self.sel = np.ascontiguousarray(sel.transpose(0, 2, 1, 3))

    def gather_core(self, table, c):
        """msgs = table[srcs[c]] staged [128, nch, F]; pads -> 0."""
        tab = np.concatenate(
            [np.zeros((1, table.shape[1]), table.dtype), table], axis=0
        )
        m = tab[self.srcs[c] + 1]                          # [nch*128, F]
        m = m.reshape(self.nch, P, -1).transpose(1, 0, 2)
        return np.ascontiguousarray(m)


# ---------------------------------------------------------------- kernels

def build_A(nc):
    """xw^T = [W1|W3]^T @ x^T, written as two feature-half tables
    out0/out1 [128, NLOCP] fp8 (host transposes for the row gather)."""
    xT = nc.dram_tensor("xT", [NFEAT, NLOCP], f16, kind="ExternalInput").ap()
    w13 = nc.dram_tensor("w13", [NFEAT, 2 * NHID], f16, kind="ExternalInput").ap()
    out0 = nc.dram_tensor("out0", [P, NLOCP], f8, kind="ExternalOutput").ap()
    out1 = nc.dram_tensor("out1", [P, NLOCP], f8, kind="ExternalOutput").ap()
    outs = [out0, out1]
    KCH = NFEAT // P              # 4
    STRIP = 3 * SUP               # 1536 cols per xT strip (SUP-aligned)
    NS = _cdiv(NLOCP, STRIP)      # 5 (last strip 256)

    with tile.TileContext(nc) as tc, ExitStack() as ctx:
        big = ctx.enter_context(tc.tile_pool(name="big", bufs=1))
        ob_pool = ctx.enter_context(tc.tile_pool(name="obp", bufs=2))
        psum = ctx.enter_context(tc.tile_pool(name="ps", bufs=4, space="PSUM"))

        w_t = big.tile([P, KCH, 2 * NHID], f16, tag="w")
        nc.sync.dma_start(
            w_t[:], w13.rearrange("(k p) f -> p k f", p=P)[:]
        )
        xts = []
        for s in range(NS):
            c0 = s * STRIP
            cw = min(STRIP, NLOCP - c0)
            per_k = []
            for k in range(KCH):
                t = big.tile([P, STRIP], f16, tag=f"xts{s}_{k}")
                nc.sync.dma_start(
                    t[:, :cw], xT[k * P:(k + 1) * P, c0:c0 + cw]
                )
                per_k.append(t)
            xts.append(per_k)

        for s in range(NS):
            c0 = s * STRIP
            cw = min(STRIP, NLOCP - c0)
            ob0 = ob_pool.tile([P, STRIP], f8, tag="ob0")
            ob1 = ob_pool.tile([P, STRIP], f8, tag="ob1")
            obs = [ob0, ob1]
            for sup in range(_cdiv(cw, SUP)):
                lc = sup * SUP
                ncols = min(SUP, cw - lc)
                for half in (0, 1):
                    ps = psum.tile([P, SUP], f32, tag="ps")
                    for k in range(KCH):
                        nc.tensor.matmul(
                            ps[:, :ncols],
                            lhsT=w_t[:, k, half * P:(half + 1) * P],
                            rhs=xts[s][k][:, lc:lc + ncols],
                            start=(k == 0), stop=(k == KCH - 1),
                        )
                    nc.vector.tensor_copy(
                        obs[half][:, lc:lc + ncols], ps[:, :ncols]
                    )
            for half in (0, 1):
                nc.sync.dma_start(outs[half][:, c0:c0 + cw], obs[half][:, :cw])
    nc.compile()
    return nc


def _sup_windows(sup):
    w0 = sup * (SUP // W)
    w1 = min(NWIN, w0 + SUP // W)
    return w0, w1


def _spmm_supertile(nc, plan, msgs_t, sel_t, ps, sup, g0):
    """Accumulate all chunks of supertile `sup` into psum tile ps."""
    w0, w1 = _sup_windows(sup)
    for w in range(w0, w1):
        lo, hi = int(plan.cs[w]), int(plan.cs[w + 1])
        for k, ch in enumerate(range(lo, hi)):
            nc.tensor.matmul(
                ps[:, (w - w0) * W:(w - w0 + 1) * W],
                lhsT=msgs_t[:, ch - g0, :],
                rhs=sel_t[:, ch - g0, :],
                start=(k == 0), stop=(k == hi - lo - 1),
            )


BUILD_SEL_B = 6      # tower-1 supertiles < this build sel on DVE


def build_B(nc, plans):
    nch = [p.nch for p in plans]
    msgs_d = [
        nc.dram_tensor(f"msgs{tw}", [P, nch[tw], NHID], f8,
                       kind="ExternalInput").ap()
        for tw in (0, 1)
    ]
    sel_d = [
        nc.dram_tensor(f"sel{tw}", [P, nch[tw], W], f16,
                       kind="ExternalInput").ap()
        for tw in (0, 1)
    ]
    dl_d = [
        nc.dram_tensor(f"dl{tw}", [P, nch[tw]], f32, kind="ExternalInput").ap()
        for tw in (0, 1)
    ]
    vl_d = [
        nc.dram_tensor(f"vl{tw}", [P, nch[tw]], f32, kind="ExternalInput").ap()
        for tw in (0, 1)
    ]
    w24 = nc.dram_tensor("w24", [NHID, 2 * NCLASS], f16, kind="ExternalInput").ap()
    b13 = nc.dram_tensor("b13", [NHID, 2], f32, kind="ExternalInput").ap()
    out = nc.dram_tensor("out", [P, NT128, 2 * NCLASS], f16,
                         kind="ExternalOutput").ap()

    with tile.TileContext(nc) as tc, ExitStack() as ctx:
        consts = ctx.enter_context(tc.tile_pool(name="consts", bufs=1))
        w24_t = consts.tile([NHID, 2 * NCLASS], f16)
        nc.sync.dma_start(w24_t[:], w24[:])
        b13_t = consts.tile([NHID, 2], f32)
        nc.sync.dma_start(b13_t[:], b13[:])
        ob = consts.tile([P, NT128, 2 * NCLASS], f16, tag="ob")
        it32 = consts.tile([P, W], mybir.dt.int32, tag="it32")
        nc.gpsimd.iota(it32[:], pattern=[[1, W]], base=0, channel_multiplier=0)
        iota = consts.tile([P, W], f16, tag="iota")
        nc.vector.tensor_copy(iota[:], it32[:])
        dl_t, vl_t = {}, {}
        for tw in (0, 1):
            t = consts.tile([P, nch[tw]], f32, tag=f"dl{tw}")
            nc.sync.dma_start(t[:], dl_d[tw][:])
            dl_t[tw] = t
            t = consts.tile([P, nch[tw]], f32, tag=f"vl{tw}")
            nc.sync.dma_start(t[:], vl_d[tw][:])
            vl_t[tw] = t

        gmax = max(
            int(p.cs[_sup_windows(s)[1]] - p.cs[_sup_windows(s)[0]])
            for p in plans for s in range(NSUP)
        )
        mpool = ctx.enter_context(tc.tile_pool(name="msgs", bufs=6))
        spool = ctx.enter_context(tc.tile_pool(name="sel", bufs=7))
        hpool = ctx.enter_context(tc.tile_pool(name="h", bufs=2))
        psum = ctx.enter_context(tc.tile_pool(name="ps", bufs=2, space="PSUM"))
        psum2 = ctx.enter_context(tc.tile_pool(name="ps2", bufs=2, space="PSUM"))

        units = []
        for tw in (0, 1):
            plan = plans[tw]
            for sup in range(NSUP):
                w0, w1 = _sup_windows(sup)
                units.append((tw, sup, int(plan.cs[w0]), int(plan.cs[w1]),
                              (w1 - w0) * W))

        def sel_prep(i):
            tw, sup, g0, g1, ncols = units[i]
            sel_t = spool.tile([P, gmax, W], f16, tag="s")
            if tw == 1 and sup in (0, 1, 7, 8):
                eng = nc.gpsimd          # Pool engine is otherwise idle
            elif tw == 0 or sup < BUILD_SEL_B + 1 or sup == 9:
                eng = nc.vector
            else:
                nc.sync.dma_start(
                    sel_t[:, :g1 - g0, :], sel_d[tw][:, g0:g1, :]
                )
                return sel_t
            for ch in range(g0, g1):
                eng.tensor_scalar(
                    out=sel_t[:, ch - g0, :], in0=iota[:],
                    scalar1=dl_t[tw][:, ch:ch + 1],
                    scalar2=vl_t[tw][:, ch:ch + 1],
                    op0=ALU.is_equal, op1=ALU.mult,
                )
            return sel_t

        sel_ready = [sel_prep(0), sel_prep(1), sel_prep(2)]
        for i, (tw, sup, g0, g1, ncols) in enumerate(units):
            plan = plans[tw]
            msgs_t = mpool.tile([P, gmax, NHID], f8, tag="m")
            nc.sync.dma_start(msgs_t[:, :g1 - g0, :], msgs_d[tw][:, g0:g1, :])
            sel_t = sel_ready.pop(0)
            if i + 3 < len(units):
                sel_ready.append(sel_prep(i + 3))

            ps = psum.tile([NHID, SUP], f32, tag="ps")
            _spmm_supertile(nc, plan, msgs_t, sel_t, ps, sup, g0)

            # h = relu(ps + b) in f16, then h @ W2 per 128-dst slice
            hT = hpool.tile([NHID, SUP], f16, tag="hT")
            nc.scalar.activation(
                out=hT[:, :ncols], in_=ps[:, :ncols], func=ACT.Relu,
                bias=b13_t[:, tw:tw + 1], scale=1.0,
            )
            for j in range(ncols // P):
                ps2 = psum2.tile([P, NCLASS], f32, tag="ps2")
                nc.tensor.matmul(
                    ps2[:], lhsT=hT[:, j * P:(j + 1) * P],
                    rhs=w24_t[:, tw * NCLASS:(tw + 1) * NCLASS],
                    start=True, stop=True,
                )
                t128 = sup * (SUP // P) + j
                nc.scalar.copy(
                    ob[:, t128, tw * NCLASS:(tw + 1) * NCLASS], ps2[:]
                )
        nc.sync.dma_start(out[:], ob[:])
    nc.compile()
    return nc


def build_C(nc, plans):
    nch = [p.nch for p in plans]
    msgs_d = [
        nc.dram_tensor(f"msgs{tw}", [P, nch[tw], NCLASS], f16,
                       kind="ExternalInput").ap()
        for tw in (0, 1)
    ]
    sel_d = [
        nc.dram_tensor(f"sel{tw}", [P, nch[tw], W], f16,
                       kind="ExternalInput").ap()
        for tw in (0, 1)
    ]
    wl = nc.dram_tensor("wl", [NCLASS, 2 * NCLASS], f16, kind="ExternalInput").ap()
    bias = nc.dram_tensor("bias", [NCLASS, 3], f32, kind="ExternalInput").ap()
    dl_d = [
        nc.dram_tensor(f"dl{tw}", [P, nch[tw]], f32, kind="ExternalInput").ap()
        for tw in (0, 1)
    ]
    vl_d = [
        nc.dram_tensor(f"vl{tw}", [P, nch[tw]], f32, kind="ExternalInput").ap()
        for tw in (0, 1)
    ]
    out = nc.dram_tensor("out", [P, NT128, NCLASS], f32,
                         kind="ExternalOutput").ap()

    with tile.TileContext(nc) as tc, ExitStack() as ctx:
        consts = ctx.enter_context(tc.tile_pool(name="consts", bufs=1))
        wl_t = consts.tile([NCLASS, 2 * NCLASS], f16)
        nc.sync.dma_start(wl_t[:], wl[:])
        bias_t = consts.tile([NCLASS, 3], f32)   # cols: b2, b4, bl
        nc.sync.dma_start(bias_t[:], bias[:])
        it32 = consts.tile([P, W], mybir.dt.int32, tag="it32")
        nc.gpsimd.iota(it32[:], pattern=[[1, W]], base=0, channel_multiplier=0)
        iota = consts.tile([P, W], f16, tag="iota")
        nc.vector.tensor_copy(iota[:], it32[:])
        dl_t, vl_t = {}, {}
        for tw in (0, 1):
            t = consts.tile([P, nch[tw]], f32, tag=f"dl{tw}")
            nc.sync.dma_start(t[:], dl_d[tw][:])
            dl_t[tw] = t
            t = consts.tile([P, nch[tw]], f32, tag=f"vl{tw}")
            nc.sync.dma_start(t[:], vl_d[tw][:])
            vl_t[tw] = t
        identf = consts.tile([NCLASS, NCLASS], f16, tag="identf")
        ident32 = consts.tile([P, P], f32, tag="ident32")
        make_identity(nc, ident32[:])
        nc.vector.tensor_copy(identf[:], ident32[0:NCLASS, 0:NCLASS])
        ob = consts.tile([P, NT128, NCLASS], f32, tag="ob")
        oT0 = consts.tile([NCLASS, NLOCP], f16, tag="oT0")
        oT1 = consts.tile([NCLASS, NLOCP], f16, tag="oT1")
        oT = [oT0, oT1]
        t_all = consts.tile([P, NT128, NCLASS], f16, tag="t_all")
        negmax_all = consts.tile([P, NT128], f32, tag="negmax")
        esum_all = consts.tile([P, NT128], f32, tag="esum")
        lse_all = consts.tile([P, NT128], f32, tag="lse")

        mpool = ctx.enter_context(tc.tile_pool(name="msgs", bufs=6))
        spool = ctx.enter_context(tc.tile_pool(name="sel", bufs=6))
        work = ctx.enter_context(tc.tile_pool(name="work", bufs=6))
        psum = ctx.enter_context(tc.tile_pool(name="ps", bufs=3, space="PSUM"))
        psumg = ctx.enter_context(tc.tile_pool(name="psg", bufs=2, space="PSUM"))
        psum2 = ctx.enter_context(tc.tile_pool(name="ps2", bufs=2, space="PSUM"))

        sup_outs = []

        def emit_softmax(sup, outT, ncols, c0):
            nblk = ncols // P
            t0 = sup * (SUP // P)
            ps_t4 = psum2.tile([P, 4, NCLASS], f16, tag="pst")
            for j in range(nblk):
                nc.tensor.transpose(
                    out=ps_t4[:, j, :], in_=outT[:, j * P:(j + 1) * P],
                    identity=identf[:],
                )
            nc.vector.tensor_reduce(
                out=negmax_all[:, t0:t0 + nblk], in_=ps_t4[:, :nblk, :],
                axis=mybir.AxisListType.X, op=ALU.max, negate=True,
            )
            for j in range(nblk):
                etmp = work.tile([P, NCLASS], f16, tag="etmp")
                nc.scalar.activation(
                    out=etmp[:], in_=ps_t4[:, j, :], func=ACT.Exp,
                    bias=negmax_all[:, t0 + j:t0 + j + 1], scale=1.0,
                    accum_out=esum_all[:, t0 + j:t0 + j + 1],
                )
            nc.vector.tensor_copy(t_all[:, t0:t0 + nblk, :], ps_t4[:, :nblk, :])

        # ---- per supertile: spmm both towers, then fused gate+softmax
        gmax = max(
            int(p.cs[_sup_windows(s)[1]] - p.cs[_sup_windows(s)[0]])
            for p in plans for s in range(NSUP)
        )
        units = []
        for sup in range(NSUP):
            w0, w1 = _sup_windows(sup)
            for tw in (0, 1):
                plan = plans[tw]
                units.append((tw, sup, int(plan.cs[w0]), int(plan.cs[w1])))

        def sel_prep(i):
            tw, sup, g0, g1 = units[i]
            sel_t = spool.tile([P, gmax, W], f16, tag="s")
            if tw == 1 and sup in (0, 1):
                eng = nc.gpsimd          # Pool engine is otherwise idle
            elif tw == 0 and sup < 7:
                eng = nc.vector
            else:
                nc.sync.dma_start(
                    sel_t[:, :g1 - g0, :], sel_d[tw][:, g0:g1, :]
                )
                return sel_t
            for ch in range(g0, g1):
                eng.tensor_scalar(
                    out=sel_t[:, ch - g0, :], in0=iota[:],
                    scalar1=dl_t[tw][:, ch:ch + 1],
                    scalar2=vl_t[tw][:, ch:ch + 1],
                    op0=ALU.is_equal, op1=ALU.mult,
                )
            return sel_t

        sel_ready = [sel_prep(0), sel_prep(1)]
        for sup in range(NSUP):
            w0, w1 = _sup_windows(sup)
            ncols = (w1 - w0) * W
            c0 = sup * SUP
            for tw in (0, 1):
                i = sup * 2 + tw
                plan = plans[tw]
                g0, g1 = int(plan.cs[w0]), int(plan.cs[w1])
                msgs_t = mpool.tile([P, gmax, NCLASS], f16, tag="m")
                nc.sync.dma_start(msgs_t[:, :g1 - g0, :], msgs_d[tw][:, g0:g1, :])
                sel_t = sel_ready.pop(0)
                if i + 2 < len(units):
                    sel_ready.append(sel_prep(i + 2))

                ps = psum.tile([NCLASS, SUP], f32, tag="ps")
                _spmm_supertile(nc, plan, msgs_t, sel_t, ps, sup, g0)
                if tw == 0:
                    nc.scalar.activation(
                        out=oT[tw][:, c0:c0 + ncols], in_=ps[:, :ncols],
                        func=ACT.Identity, bias=bias_t[:, tw:tw + 1], scale=1.0,
                    )
                else:
                    nc.vector.tensor_scalar(
                        out=oT[tw][:, c0:c0 + ncols],
                        in0=ps[:, :ncols],
                        scalar1=bias_t[:, tw:tw + 1], scalar2=None, op0=ALU.add,
                    )
            ps_g = psumg.tile([NCLASS, SUP], f32, tag="psg")
            nc.tensor.matmul(
                ps_g[:, :ncols], lhsT=wl_t[:, 0:NCLASS],
                rhs=oT[0][:, c0:c0 + ncols],
                start=True, stop=False,
            )
            nc.tensor.matmul(
                ps_g[:, :ncols], lhsT=wl_t[:, NCLASS:2 * NCLASS],
                rhs=oT[1][:, c0:c0 + ncols],
                start=False, stop=True,
            )
            if sup == NSUP - 1:
                # early finish for blocks of sups 0..10: their esums are
                # ready; the Ln table reload and the 44 combines hide
                # under this last supertile's stream
                nfin = (NSUP - 2) * (SUP // P)
                nc.scalar.activation(
                    out=lse_all[:, 0:nfin], in_=esum_all[:, 0:nfin],
                    func=ACT.Ln,
                )
                for t128 in range(nfin):
                    nc.vector.tensor_scalar(
                        out=ob[:, t128, :], in0=t_all[:, t128, :],
                        scalar1=negmax_all[:, t128:t128 + 1],
                        scalar2=lse_all[:, t128:t128 + 1],
                        op0=ALU.add, op1=ALU.subtract,
                    )
                nc.sync.dma_start(out[:, 0:nfin, :], ob[:, 0:nfin, :])

            # gate = 1 / (1 + exp(-(z + bl))); bias col 2 holds -bl
            eneg = work.tile([NCLASS, SUP], f32, tag="eneg")
            nc.scalar.activation(
                out=eneg[:, :ncols], in_=ps_g[:, :ncols], func=ACT.Exp,
                bias=bias_t[:, 2:3], scale=-1.0,
            )
            # softmax of the previous supertile: its inputs are ready, and
            # emitting it here keeps DVE busy while ACT computes eneg
            if len(sup_outs) > 0:
                emit_softmax(*sup_outs.pop(0))
            dif = work.tile([NCLASS, SUP], f16, tag="dif")
            nc.vector.tensor_tensor(
                out=dif[:, :ncols], in0=oT[0][:, c0:c0 + ncols],
                in1=oT[1][:, c0:c0 + ncols], op=ALU.subtract,
            )
            den = work.tile([NCLASS, SUP], f32, tag="den")
            nc.vector.tensor_scalar(
                out=den[:, :ncols], in0=eneg[:, :ncols], scalar1=1.0,
                scalar2=None, op0=ALU.add,
            )
            gt = work.tile([NCLASS, SUP], f32, tag="gt")
            nc.vector.reciprocal(gt[:, :ncols], den[:, :ncols])
            nc.vector.tensor_tensor(out=dif[:, :ncols], in0=gt[:, :ncols],
                                    in1=dif[:, :ncols], op=ALU.mult)
            outT = work.tile([NCLASS, SUP], f16, tag="outT")
            nc.vector.tensor_tensor(
                out=outT[:, :ncols], in0=oT[1][:, c0:c0 + ncols],
                in1=dif[:, :ncols], op=ALU.add,
            )
            sup_outs.append((sup, outT, ncols, c0))
        emit_softmax(*sup_outs.pop(0))

        nfin = (NSUP - 2) * (SUP // P)
        nc.scalar.activation(out=lse_all[:, nfin:NT128],
                             in_=esum_all[:, nfin:NT128], func=ACT.Ln)
        for t128 in range(nfin, NT128):
            nc.vector.tensor_scalar(
                out=ob[:, t128, :], in0=t_all[:, t128, :],
                scalar1=negmax_all[:, t128:t128 + 1],
                scalar2=lse_all[:, t128:t128 + 1],
                op0=ALU.add, op1=ALU.subtract,
            )
        nc.sync.dma_start(out[:, nfin:NT128, :], ob[:, nfin:NT128, :])
    nc.compile()
    return nc


# ---------------------------------------------------------------- driver

TRACE = False          # set by test.py to collect per-launch artifacts
LAST_NCS = []          # built Bass modules per launch when TRACE


def _run(nc, in_maps):
    if TRACE:
        LAST_NCS.append(nc)
    return run_bass_kernel_spmd(nc, in_maps, core_ids=list(range(NCORES)))


def _make_nc():
    return bacc.Bacc(
        "TRN2", target_bir_lowering=False, debug=False,
        num_devices=NCORES, num_swdge_queues=1,
    )


def kernel(x, edge_index, edge_vals, edge_index2, edge_vals2,
           W1, b1, W2, b2, W3, b3, W4, b4, Wl, bl):
    x = np.asarray(x, np.float32).astype(np.float16)
    degs = [np.bincount(np.asarray(ei[1]).astype(np.int64), minlength=N)
            for ei in (edge_index, edge_index2)]
    row_of = balance_rows(degs)
    plans = [TowerPlan(edge_index, edge_vals, row_of),
             TowerPlan(edge_index2, edge_vals2, row_of)]

    # ---- launch A: xw = x @ [W1|W3]  (fp8 table out)
    w13 = np.concatenate([np.asarray(W1, np.float32),
                          np.asarray(W3, np.float32)], axis=1).astype(np.float16)
    nc = _make_nc()
    build_A(nc)
    in_maps = []
    for c in range(NCORES):
        xT = np.zeros((NFEAT, NLOCP), np.float16)
        rows = row_of[c * NLOC:(c + 1) * NLOC]
        xT[:, rows] = x[c * NLOC:(c + 1) * NLOC].T
        in_maps.append({"xT": xT, "w13": w13})
    res = _run(nc, in_maps)
    xw = np.zeros((N, 2 * NHID), F8NP)
    for c in range(NCORES):
        rows = row_of[c * NLOC:(c + 1) * NLOC]
        xw[c * NLOC:(c + 1) * NLOC, 0:NHID] = \
            np.asarray(res.results[c]["out0"]).T[rows]
        xw[c * NLOC:(c + 1) * NLOC, NHID:2 * NHID] = \
            np.asarray(res.results[c]["out1"]).T[rows]

    # ---- launch B: h = relu(spmm(xw) + b); hw2 = h @ [W2|W4]
    w24 = np.concatenate([np.asarray(W2, np.float32),
                          np.asarray(W4, np.float32)], axis=1).astype(np.float16)
    b13 = np.stack([np.asarray(b1, np.float32),
                    np.asarray(b3, np.float32)], axis=1)
    nc = _make_nc()
    build_B(nc, plans)
    in_maps = []
    for c in range(NCORES):
        m = {"w24": w24, "b13": b13}
        for tw in (0, 1):
            m[f"msgs{tw}"] = plans[tw].gather_core(
                xw[:, tw * NHID:(tw + 1) * NHID], c)
            m[f"sel{tw}"] = plans[tw].sel[c]
            m[f"dl{tw}"] = plans[tw].dl[c]
            m[f"vl{tw}"] = plans[tw].vl[c]
        in_maps.append(m)
    res = _run(nc, in_maps)
    hw2 = np.zeros((N, 2 * NCLASS), np.float16)
    for c in range(NCORES):
        full = np.asarray(res.results[c]["out"]).transpose(1, 0, 2)
        rows = row_of[c * NLOC:(c + 1) * NLOC]
        hw2[c * NLOC:(c + 1) * NLOC] = full.reshape(NLOCP, 2 * NCLASS)[rows]

    # ---- launch C: o = spmm(hw2) + b; gated fusion; log_softmax
    wl_f = np.asarray(Wl, np.float32)
    wl = np.concatenate([wl_f[0:NCLASS], wl_f[NCLASS:2 * NCLASS]],
                        axis=1).astype(np.float16)
    bias = np.stack([np.asarray(b2, np.float32),
                     np.asarray(b4, np.float32),
                     -np.asarray(bl, np.float32)], axis=1)
    nc = _make_nc()
    build_C(nc, plans)
    in_maps = []
    for c in range(NCORES):
        m = {"wl": wl, "bias": bias}
        for tw in (0, 1):
            m[f"msgs{tw}"] = plans[tw].gather_core(
                hw2[:, tw * NCLASS:(tw + 1) * NCLASS], c)
            m[f"sel{tw}"] = plans[tw].sel[c]
            m[f"dl{tw}"] = plans[tw].dl[c]
            m[f"vl{tw}"] = plans[tw].vl[c]
        in_maps.append(m)
    res = _run(nc, in_maps)
    out = np.zeros((N, NCLASS), np.float32)
    for c in range(NCORES):
        full = np.asarray(res.results[c]["out"]).transpose(1, 0, 2)
        rows = row_of[c * NLOC:(c + 1) * NLOC]
        out[c * NLOC:(c + 1) * NLOC] = full.reshape(NLOCP, NCLASS)[rows]
    return out


# revision 69
# speedup vs baseline: 1.0118x; 1.0118x over previous
"""Trainium2 Bass kernel for the two-tower GCN (nn_GCN2).

Distribution: nodes partitioned by destination range across 8 cores
(graph parallel). All floating-point math runs on device across 3 SPMD
launches; the host only does index manipulation (edge sorting, row
gathering by static indices, dtype casts of inputs) and the inter-launch
reshard/halo-exchange, exactly like the sharding contract allows:

  A: xw  = x @ [W1|W3]                 (node-sharded dense matmul, fp8 out)
  B: h   = relu(spmm(A, xw) + b); hw2 = h @ [W2|W4]    (per dst window)
  C: o   = spmm(A, hw2) + b; gated fusion; log_softmax (per dst window)

The irregular gather of source features is resolved on the host between
launches: since the edge list is static, the per-edge message stream
msgs[chunk, slot, :] = table[src[chunk, slot]] is a pure row-gather of
the previous launch's output, staged partition-major so the device
streams it at full contiguous-DMA bandwidth. The segment-sum runs on the
tensor engine: edges are sorted by destination, so each chunk of 128
edges lands in one 32-wide destination window and
psum[:, win] += msgs_chunk^T @ sel_chunk with a host-built fp8 selector
sel[slot, d] = val * (dst_local == d).
"""
from contextlib import ExitStack

import numpy as np

import concourse.bass as bass
import concourse.tile as tile
from concourse import bacc, mybir
from concourse.bass_utils import run_bass_kernel_spmd
from concourse.masks import make_identity

P = 128
NCORES = 8
N = 50000
E = 800000
NFEAT = 512
NHID = 128
NCLASS = 40
NLOC = N // NCORES             # 6250 real nodes per core
NT128 = 50                     # 128-row blocks per core
NLOCP = NT128 * P              # 6400 padded rows per core (slack for packing)
W = 32                         # dst window width (sel columns)
NWIN = NLOCP // W              # 200 windows per core
SUP = 512                      # dsts per PSUM supertile
NSUP = (NLOCP + SUP - 1) // SUP  # 13 (last one is 256 wide)

f16 = mybir.dt.float16
f32 = mybir.dt.float32
f8 = mybir.dt.float8e4
ACT = mybir.ActivationFunctionType
ALU = mybir.AluOpType
F8NP = mybir.dt.np(f8)


def _cdiv(a, b):
    return (a + b - 1) // b


# ---------------------------------------------------------------- host prep

def balance_rows(degs):
    """Assign each core's nodes to 32-slot windows so that both towers'
    per-window edge counts stay <= 4*128 (pure index manipulation).

    Returns row_of: [N] padded row index of each node within its core.
    """
    deg0, deg1 = degs
    row_of = np.empty(N, np.int64)
    for c in range(NCORES):
        nodes = np.arange(c * NLOC, (c + 1) * NLOC)
        nodes = nodes[np.argsort(-(deg0[nodes] + deg1[nodes]), kind="stable")]
        s0 = np.zeros(NWIN)
        s1 = np.zeros(NWIN)
        cnt = np.zeros(NWIN, np.int64)
        for nd in nodes:
            m = np.where(cnt < 32,
                         np.maximum(s0 + deg0[nd], s1 + deg1[nd]), 1e18)
            w = int(np.argmin(m))
            row_of[nd] = w * W + cnt[w]
            s0[w] += deg0[nd]
            s1[w] += deg1[nd]
            cnt[w] += 1
    return row_of


class TowerPlan:
    """Edge preprocessing for one tower (one graph).

    Sorts each core's in-edges by destination window (32-wide), splits
    them into chunks of 128 slots, pads every (window) to the max chunk
    count over cores so all cores run one program, and records per-slot
    (src, dst_col, val).

    Produces:
      nch          : padded chunk count (same for all cores)
      cs           : [NWIN+1] chunk range per window
      srcs         : [NCORES, nch*128] int32 source row, -1 for pads
      sel          : [NCORES, 128, nch, W] f16 selector (val at dst col)
    """

    def __init__(self, edge_index, edge_vals, row_of):
        src = np.asarray(edge_index[0]).astype(np.int64)
        dst = np.asarray(edge_index[1]).astype(np.int64)
        vals = np.asarray(edge_vals).astype(np.float32)

        core = dst // NLOC
        ldst = row_of[dst]
        win = ldst // W
        col = ldst - win * W

        counts = np.zeros((NCORES, NWIN), np.int64)
        np.add.at(counts, (core, win), 1)
        chunk_cnt = np.maximum(_cdiv(counts, P).max(axis=0), 1)  # [NWIN]
        self.cs = np.concatenate([[0], np.cumsum(chunk_cnt)])
        self.nch = int(self.cs[-1])

        order = np.lexsort((ldst, win, core))
        so_core, so_win = core[order], win[order]
        so_src, so_col, so_val = src[order], col[order], vals[order]
        gkey = so_core * NWIN + so_win
        gstart = np.r_[0, np.flatnonzero(np.diff(gkey)) + 1]
        glen = np.diff(np.r_[gstart, len(gkey)])
        rank = np.arange(len(gkey)) - np.repeat(gstart, glen)
        slot = (self.cs[so_win] * P + rank).astype(np.int64)

        nslot = self.nch * P
        self.srcs = np.full((NCORES, nslot), -1, np.int32)
        cola = np.zeros((NCORES, nslot), np.int64)
        vala = np.zeros((NCORES, nslot), np.float32)
        flat = so_core * nslot + slot
        self.srcs.reshape(-1)[flat] = so_src.astype(np.int32)
        cola.reshape(-1)[flat] = so_col
        vala.reshape(-1)[flat] = so_val

        # compact per-slot forms for on-device sel builds
        self.dl = np.ascontiguousarray(
            cola.reshape(NCORES, self.nch, P).transpose(0, 2, 1)
        ).astype(np.float32)
        self.vl = np.ascontiguousarray(
            vala.reshape(NCORES, self.nch, P).transpose(0, 2, 1)
        ).astype(np.float32)

        # sel[c, p, ch, col] = val of slot (ch, p) if its dst col matches
        sel = np.zeros((NCORES, self.nch, P, W), np.float16)
        ci = np.arange(nslot) // P
        pi = np.arange(nslot) % P
        v8 = vala.astype(np.float16)
        for c in range(NCORES):
            sel[c, ci, pi, cola[c]] = v8[c]
            # pads: col 0 with val 0 already zero
        self.sel = np.ascontiguousarray(sel.transpose(0, 2, 1, 3))

    def gather_core(self, table, c):
        """msgs = table[srcs[c]] staged [128, nch, F]; pads -> 0."""
        tab = np.concatenate(
            [np.zeros((1, table.shape[1]), table.dtype), table], axis=0
        )
        m = tab[self.srcs[c] + 1]                          # [nch*128, F]
        m = m.reshape(self.nch, P, -1).transpose(1, 0, 2)
        return np.ascontiguousarray(m)


# ---------------------------------------------------------------- kernels

def build_A(nc):
    """xw^T = [W1|W3]^T @ x^T, written as two feature-half tables
    out0/out1 [128, NLOCP] fp8 (host transposes for the row gather)."""
    xT = nc.dram_tensor("xT", [NFEAT, NLOCP], f16, kind="ExternalInput").ap()
    w13 = nc.dram_tensor("w13", [NFEAT, 2 * NHID], f16, kind="ExternalInput").ap()
    out0 = nc.dram_tensor("out0", [P, NLOCP], f8, kind="ExternalOutput").ap()
    out1 = nc.dram_tensor("out1", [P, NLOCP], f8, kind="ExternalOutput").ap()
    outs = [out0, out1]
    KCH = NFEAT // P              # 4
    STRIP = 3 * SUP               # 1536 cols per xT strip (SUP-aligned)
    NS = _cdiv(NLOCP, STRIP)      # 5 (last strip 256)

    with tile.TileContext(nc) as tc, ExitStack() as ctx:
        big = ctx.enter_context(tc.tile_pool(name="big", bufs=1))
        ob_pool = ctx.enter_context(tc.tile_pool(name="obp", bufs=2))
        psum = ctx.enter_context(tc.tile_pool(name="ps", bufs=4, space="PSUM"))

        w_t = big.tile([P, KCH, 2 * NHID], f16, tag="w")
        nc.sync.dma_start(
            w_t[:], w13.rearrange("(k p) f -> p k f", p=P)[:]
        )
        xts = []
        for s in range(NS):
            c0 = s * STRIP
            cw = min(STRIP, NLOCP - c0)
            per_k = []
            for k in range(KCH):
                t = big.tile([P, STRIP], f16, tag=f"xts{s}_{k}")
                nc.sync.dma_start(
                    t[:, :cw], xT[k * P:(k + 1) * P, c0:c0 + cw]
                )
                per_k.append(t)
            xts.append(per_k)

        for s in range(NS):
            c0 = s * STRIP
            cw = min(STRIP, NLOCP - c0)
            ob0 = ob_pool.tile([P, STRIP], f8, tag="ob0")
            ob1 = ob_pool.tile([P, STRIP], f8, tag="ob1")
            obs = [ob0, ob1]
            for sup in range(_cdiv(cw, SUP)):
                lc = sup * SUP
                ncols = min(SUP, cw - lc)
                for half in (0, 1):
                    ps = psum.tile([P, SUP], f32, tag="ps")
                    for k in range(KCH):
                        nc.tensor.matmul(
                            ps[:, :ncols],
                            lhsT=w_t[:, k, half * P:(half + 1) * P],
                            rhs=xts[s][k][:, lc:lc + ncols],
                            start=(k == 0), stop=(k == KCH - 1),
                        )
                    nc.vector.tensor_copy(
                        obs[half][:, lc:lc + ncols], ps[:, :ncols]
                    )
            for half in (0, 1):
                nc.sync.dma_start(outs[half][:, c0:c0 + cw], obs[half][:, :cw])
    nc.compile()
    return nc


def _sup_windows(sup):
    w0 = sup * (SUP // W)
    w1 = min(NWIN, w0 + SUP // W)
    return w0, w1


def _spmm_supertile(nc, plan, msgs_t, sel_t, ps, sup, g0):
    """Accumulate all chunks of supertile `sup` into psum tile ps."""
    w0, w1 = _sup_windows(sup)
    for w in range(w0, w1):
        lo, hi = int(plan.cs[w]), int(plan.cs[w + 1])
        for k, ch in enumerate(range(lo, hi)):
            nc.tensor.matmul(
                ps[:, (w - w0) * W:(w - w0 + 1) * W],
                lhsT=msgs_t[:, ch - g0, :],
                rhs=sel_t[:, ch - g0, :],
                start=(k == 0), stop=(k == hi - lo - 1),
            )


BUILD_SEL_B = 6      # tower-1 supertiles < this build sel on DVE


def build_B(nc, plans):
    nch = [p.nch for p in plans]
    msgs_d = [
        nc.dram_tensor(f"msgs{tw}", [P, nch[tw], NHID], f8,
                       kind="ExternalInput").ap()
        for tw in (0, 1)
    ]
    sel_d = [
        nc.dram_tensor(f"sel{tw}", [P, nch[tw], W], f16,
                       kind="ExternalInput").ap()
        for tw in (0, 1)
    ]
    dl_d = [
        nc.dram_tensor(f"dl{tw}", [P, nch[tw]], f32, kind="ExternalInput").ap()
        for tw in (0, 1)
    ]
    vl_d = [
        nc.dram_tensor(f"vl{tw}", [P, nch[tw]], f32, kind="ExternalInput").ap()
        for tw in (0, 1)
    ]
    w24 = nc.dram_tensor("w24", [NHID, 2 * NCLASS], f16, kind="ExternalInput").ap()
    b13 = nc.dram_tensor("b13", [NHID, 2], f32, kind="ExternalInput").ap()
    out = nc.dram_tensor("out", [P, NT128, 2 * NCLASS], f16,
                         kind="ExternalOutput").ap()

    with tile.TileContext(nc) as tc, ExitStack() as ctx:
        consts = ctx.enter_context(tc.tile_pool(name="consts", bufs=1))
        w24_t = consts.tile([NHID, 2 * NCLASS], f16)
        nc.sync.dma_start(w24_t[:], w24[:])
        b13_t = consts.tile([NHID, 2], f32)
        nc.sync.dma_start(b13_t[:], b13[:])
        ob = consts.tile([P, NT128, 2 * NCLASS], f16, tag="ob")
        it32 = consts.tile([P, W], mybir.dt.int32, tag="it32")
        nc.gpsimd.iota(it32[:], pattern=[[1, W]], base=0, channel_multiplier=0)
        iota = consts.tile([P, W], f16, tag="iota")
        nc.vector.tensor_copy(iota[:], it32[:])
        dl_t, vl_t = {}, {}
        for tw in (0, 1):
            t = consts.tile([P, nch[tw]], f32, tag=f"dl{tw}")
            nc.sync.dma_start(t[:], dl_d[tw][:])
            dl_t[tw] = t
            t = consts.tile([P, nch[tw]], f32, tag=f"vl{tw}")
            nc.sync.dma_start(t[:], vl_d[tw][:])
            vl_t[tw] = t

        gmax = max(
            int(p.cs[_sup_windows(s)[1]] - p.cs[_sup_windows(s)[0]])
            for p in plans for s in range(NSUP)
        )
        mpool = ctx.enter_context(tc.tile_pool(name="msgs", bufs=6))
        spool = ctx.enter_context(tc.tile_pool(name="sel", bufs=7))
        hpool = ctx.enter_context(tc.tile_pool(name="h", bufs=2))
        psum = ctx.enter_context(tc.tile_pool(name="ps", bufs=2, space="PSUM"))
        psum2 = ctx.enter_context(tc.tile_pool(name="ps2", bufs=2, space="PSUM"))

        units = []
        for tw in (0, 1):
            plan = plans[tw]
            for sup in range(NSUP):
                w0, w1 = _sup_windows(sup)
                units.append((tw, sup, int(plan.cs[w0]), int(plan.cs[w1]),
                              (w1 - w0) * W))

        def sel_prep(i):
            tw, sup, g0, g1, ncols = units[i]
            sel_t = spool.tile([P, gmax, W], f16, tag="s")
            if tw == 1 and sup in (0, 1, 7, 8, 10):
                eng = nc.gpsimd          # Pool engine is otherwise idle
            elif tw == 0 or sup < BUILD_SEL_B + 1 or sup == 9:
                eng = nc.vector
            else:
                nc.sync.dma_start(
                    sel_t[:, :g1 - g0, :], sel_d[tw][:, g0:g1, :]
                )
                return sel_t
            for ch in range(g0, g1):
                eng.tensor_scalar(
                    out=sel_t[:, ch - g0, :], in0=iota[:],
                    scalar1=dl_t[tw][:, ch:ch + 1],
                    scalar2=vl_t[tw][:, ch:ch + 1],
                    op0=ALU.is_equal, op1=ALU.mult,
                )
            return sel_t

        sel_ready = [sel_prep(0), sel_prep(1), sel_prep(2)]
        for i, (tw, sup, g0, g1, ncols) in enumerate(units):
            plan = plans[tw]
            msgs_t = mpool.tile([P, gmax, NHID], f8, tag="m")
            nc.sync.dma_start(msgs_t[:, :g1 - g0, :], msgs_d[tw][:, g0:g1, :])
            sel_t = sel_ready.pop(0)
            if i + 3 < len(units):
                sel_ready.append(sel_prep(i + 3))

            ps = psum.tile([NHID, SUP], f32, tag="ps")
            _spmm_supertile(nc, plan, msgs_t, sel_t, ps, sup, g0)

            # h = relu(ps + b) in f16, then h @ W2 per 128-dst slice
            hT = hpool.tile([NHID, SUP], f16, tag="hT")
            nc.scalar.activation(
                out=hT[:, :ncols], in_=ps[:, :ncols], func=ACT.Relu,
                bias=b13_t[:, tw:tw + 1], scale=1.0,
            )
            for j in range(ncols // P):
                ps2 = psum2.tile([P, NCLASS], f32, tag="ps2")
                nc.tensor.matmul(
                    ps2[:], lhsT=hT[:, j * P:(j + 1) * P],
                    rhs=w24_t[:, tw * NCLASS:(tw + 1) * NCLASS],
                    start=True, stop=True,
                )
                t128 = sup * (SUP // P) + j
                nc.scalar.copy(
                    ob[:, t128, tw * NCLASS:(tw + 1) * NCLASS], ps2[:]
                )
            if tw == 1 and sup == 5:
                # blocks 0..23 are complete for both towers: stream them
                # out now so only a small output write remains at the end
                nc.sync.dma_start(out[:, 0:24, :], ob[:, 0:24, :])
            if tw == 1 and sup == 9:
                nc.sync.dma_start(out[:, 24:40, :], ob[:, 24:40, :])
            if tw == 1 and sup == 11:
                nc.sync.dma_start(out[:, 40:48, :], ob[:, 40:48, :])
        nc.sync.dma_start(out[:, 48:NT128, :], ob[:, 48:NT128, :])
    nc.compile()
    return nc


def build_C(nc, plans):
    nch = [p.nch for p in plans]
    msgs_d = [
        nc.dram_tensor(f"msgs{tw}", [P, nch[tw], NCLASS], f16,
                       kind="ExternalInput").ap()
        for tw in (0, 1)
    ]
    sel_d = [
        nc.dram_tensor(f"sel{tw}", [P, nch[tw], W], f16,
                       kind="ExternalInput").ap()
        for tw in (0, 1)
    ]
    wl = nc.dram_tensor("wl", [NCLASS, 2 * NCLASS], f16, kind="ExternalInput").ap()
    bias = nc.dram_tensor("bias", [NCLASS, 3], f32, kind="ExternalInput").ap()
    dl_d = [
        nc.dram_tensor(f"dl{tw}", [P, nch[tw]], f32, kind="ExternalInput").ap()
        for tw in (0, 1)
    ]
    vl_d = [
        nc.dram_tensor(f"vl{tw}", [P, nch[tw]], f32, kind="ExternalInput").ap()
        for tw in (0, 1)
    ]
    out = nc.dram_tensor("out", [P, NT128, NCLASS], f32,
                         kind="ExternalOutput").ap()

    with tile.TileContext(nc) as tc, ExitStack() as ctx:
        consts = ctx.enter_context(tc.tile_pool(name="consts", bufs=1))
        wl_t = consts.tile([NCLASS, 2 * NCLASS], f16)
        nc.sync.dma_start(wl_t[:], wl[:])
        bias_t = consts.tile([NCLASS, 3], f32)   # cols: b2, b4, bl
        nc.sync.dma_start(bias_t[:], bias[:])
        it32 = consts.tile([P, W], mybir.dt.int32, tag="it32")
        nc.gpsimd.iota(it32[:], pattern=[[1, W]], base=0, channel_multiplier=0)
        iota = consts.tile([P, W], f16, tag="iota")
        nc.vector.tensor_copy(iota[:], it32[:])
        dl_t, vl_t = {}, {}
        for tw in (0, 1):
            t = consts.tile([P, nch[tw]], f32, tag=f"dl{tw}")
            nc.sync.dma_start(t[:], dl_d[tw][:])
            dl_t[tw] = t
            t = consts.tile([P, nch[tw]], f32, tag=f"vl{tw}")
            nc.sync.dma_start(t[:], vl_d[tw][:])
            vl_t[tw] = t
        identf = consts.tile([NCLASS, NCLASS], f16, tag="identf")
        ident32 = consts.tile([P, P], f32, tag="ident32")
        make_identity(nc, ident32[:])
        nc.vector.tensor_copy(identf[:], ident32[0:NCLASS, 0:NCLASS])
        ob = consts.tile([P, NT128, NCLASS], f32, tag="ob")
        oT0 = consts.tile([NCLASS, NLOCP], f16, tag="oT0")
        oT1 = consts.tile([NCLASS, NLOCP], f16, tag="oT1")
        oT = [oT0, oT1]
        t_all = consts.tile([P, NT128, NCLASS], f16, tag="t_all")
        negmax_all = consts.tile([P, NT128], f32, tag="negmax")
        esum_all = consts.tile([P, NT128], f32, tag="esum")
        lse_all = consts.tile([P, NT128], f32, tag="lse")

        mpool = ctx.enter_context(tc.tile_pool(name="msgs", bufs=6))
        spool = ctx.enter_context(tc.tile_pool(name="sel", bufs=6))
        work = ctx.enter_context(tc.tile_pool(name="work", bufs=6))
        psum = ctx.enter_context(tc.tile_pool(name="ps", bufs=3, space="PSUM"))
        psumg = ctx.enter_context(tc.tile_pool(name="psg", bufs=2, space="PSUM"))
        psum2 = ctx.enter_context(tc.tile_pool(name="ps2", bufs=2, space="PSUM"))

        sup_outs = []

        def emit_softmax(sup, outT, ncols, c0):
            nblk = ncols // P
            t0 = sup * (SUP // P)
            ps_t4 = psum2.tile([P, 4, NCLASS], f16, tag="pst")
            for j in range(nblk):
                nc.tensor.transpose(
                    out=ps_t4[:, j, :], in_=outT[:, j * P:(j + 1) * P],
                    identity=identf[:],
                )
            nc.vector.tensor_reduce(
                out=negmax_all[:, t0:t0 + nblk], in_=ps_t4[:, :nblk, :],
                axis=mybir.AxisListType.X, op=ALU.max, negate=True,
            )
            for j in range(nblk):
                etmp = work.tile([P, NCLASS], f16, tag="etmp")
                nc.scalar.activation(
                    out=etmp[:], in_=ps_t4[:, j, :], func=ACT.Exp,
                    bias=negmax_all[:, t0 + j:t0 + j + 1], scale=1.0,
                    accum_out=esum_all[:, t0 + j:t0 + j + 1],
                )
            nc.vector.tensor_copy(t_all[:, t0:t0 + nblk, :], ps_t4[:, :nblk, :])

        # ---- per supertile: spmm both towers, then fused gate+softmax
        gmax = max(
            int(p.cs[_sup_windows(s)[1]] - p.cs[_sup_windows(s)[0]])
            for p in plans for s in range(NSUP)
        )
        units = []
        for sup in range(NSUP):
            w0, w1 = _sup_windows(sup)
            for tw in (0, 1):
                plan = plans[tw]
                units.append((tw, sup, int(plan.cs[w0]), int(plan.cs[w1])))

        def sel_prep(i):
            tw, sup, g0, g1 = units[i]
            sel_t = spool.tile([P, gmax, W], f16, tag="s")
            if tw == 1 and sup in (0, 1):
                eng = nc.gpsimd          # Pool engine is otherwise idle
            elif tw == 0 and sup < 7:
                eng = nc.vector
            else:
                nc.sync.dma_start(
                    sel_t[:, :g1 - g0, :], sel_d[tw][:, g0:g1, :]
                )
                return sel_t
            for ch in range(g0, g1):
                eng.tensor_scalar(
                    out=sel_t[:, ch - g0, :], in0=iota[:],
                    scalar1=dl_t[tw][:, ch:ch + 1],
                    scalar2=vl_t[tw][:, ch:ch + 1],
                    op0=ALU.is_equal, op1=ALU.mult,
                )
            return sel_t

        sel_ready = [sel_prep(0), sel_prep(1)]
        for sup in range(NSUP):
            w0, w1 = _sup_windows(sup)
            ncols = (w1 - w0) * W
            c0 = sup * SUP
            for tw in (0, 1):
                i = sup * 2 + tw
                plan = plans[tw]
                g0, g1 = int(plan.cs[w0]), int(plan.cs[w1])
                msgs_t = mpool.tile([P, gmax, NCLASS], f16, tag="m")
                nc.sync.dma_start(msgs_t[:, :g1 - g0, :], msgs_d[tw][:, g0:g1, :])
                sel_t = sel_ready.pop(0)
                if i + 2 < len(units):
                    sel_ready.append(sel_prep(i + 2))

                ps = psum.tile([NCLASS, SUP], f32, tag="ps")
                _spmm_supertile(nc, plan, msgs_t, sel_t, ps, sup, g0)
                if tw == 0:
                    nc.scalar.activation(
                        out=oT[tw][:, c0:c0 + ncols], in_=ps[:, :ncols],
                        func=ACT.Identity, bias=bias_t[:, tw:tw + 1], scale=1.0,
                    )
                else:
                    nc.vector.tensor_scalar(
                        out=oT[tw][:, c0:c0 + ncols],
                        in0=ps[:, :ncols],
                        scalar1=bias_t[:, tw:tw + 1], scalar2=None, op0=ALU.add,
                    )
            ps_g = psumg.tile([NCLASS, SUP], f32, tag="psg")
            nc.tensor.matmul(
                ps_g[:, :ncols], lhsT=wl_t[:, 0:NCLASS],
                rhs=oT[0][:, c0:c0 + ncols],
                start=True, stop=False,
            )
            nc.tensor.matmul(
                ps_g[:, :ncols], lhsT=wl_t[:, NCLASS:2 * NCLASS],
                rhs=oT[1][:, c0:c0 + ncols],
                start=False, stop=True,
            )
            if sup == NSUP - 1:
                # early finish for blocks of sups 0..10: their esums are
                # ready; the Ln table reload and the 44 combines hide
                # under this last supertile's stream
                nfin = (NSUP - 2) * (SUP // P)
                nc.scalar.activation(
                    out=lse_all[:, 0:nfin], in_=esum_all[:, 0:nfin],
                    func=ACT.Ln,
                )
                for t128 in range(nfin):
                    nc.vector.tensor_scalar(
                        out=ob[:, t128, :], in0=t_all[:, t128, :],
                        scalar1=negmax_all[:, t128:t128 + 1],
                        scalar2=lse_all[:, t128:t128 + 1],
                        op0=ALU.add, op1=ALU.subtract,
                    )
                nc.sync.dma_start(out[:, 0:nfin, :], ob[:, 0:nfin, :])

            # gate = 1 / (1 + exp(-(z + bl))); bias col 2 holds -bl
            eneg = work.tile([NCLASS, SUP], f32, tag="eneg")
            nc.scalar.activation(
                out=eneg[:, :ncols], in_=ps_g[:, :ncols], func=ACT.Exp,
                bias=bias_t[:, 2:3], scale=-1.0,
            )
            # softmax of the previous supertile: its inputs are ready, and
            # emitting it here keeps DVE busy while ACT computes eneg
            if len(sup_outs) > 0:
                emit_softmax(*sup_outs.pop(0))
            dif = work.tile([NCLASS, SUP], f16, tag="dif")
            nc.vector.tensor_tensor(
                out=dif[:, :ncols], in0=oT[0][:, c0:c0 + ncols],
                in1=oT[1][:, c0:c0 + ncols], op=ALU.subtract,
            )
            den = work.tile([NCLASS, SUP], f32, tag="den")
            nc.vector.tensor_scalar(
                out=den[:, :ncols], in0=eneg[:, :ncols], scalar1=1.0,
                scalar2=None, op0=ALU.add,
            )
            gt = work.tile([NCLASS, SUP], f32, tag="gt")
            nc.vector.reciprocal(gt[:, :ncols], den[:, :ncols])
            nc.vector.tensor_tensor(out=dif[:, :ncols], in0=gt[:, :ncols],
                                    in1=dif[:, :ncols], op=ALU.mult)
            outT = work.tile([NCLASS, SUP], f16, tag="outT")
            nc.vector.tensor_tensor(
                out=outT[:, :ncols], in0=oT[1][:, c0:c0 + ncols],
                in1=dif[:, :ncols], op=ALU.add,
            )
            sup_outs.append((sup, outT, ncols, c0))
        emit_softmax(*sup_outs.pop(0))

        nfin = (NSUP - 2) * (SUP // P)
        nc.scalar.activation(out=lse_all[:, nfin:NT128],
                             in_=esum_all[:, nfin:NT128], func=ACT.Ln)
        for t128 in range(nfin, NT128):
            nc.vector.tensor_scalar(
                out=ob[:, t128, :], in0=t_all[:, t128, :],
                scalar1=negmax_all[:, t128:t128 + 1],
                scalar2=lse_all[:, t128:t128 + 1],
                op0=ALU.add, op1=ALU.subtract,
            )
        nc.sync.dma_start(out[:, nfin:NT128, :], ob[:, nfin:NT128, :])
    nc.compile()
    return nc


# ---------------------------------------------------------------- driver

TRACE = False          # set by test.py to collect per-launch artifacts
LAST_NCS = []          # built Bass modules per launch when TRACE


def _run(nc, in_maps):
    if TRACE:
        LAST_NCS.append(nc)
    return run_bass_kernel_spmd(nc, in_maps, core_ids=list(range(NCORES)))


def _make_nc():
    return bacc.Bacc(
        "TRN2", target_bir_lowering=False, debug=False,
        num_devices=NCORES, num_swdge_queues=1,
    )


def kernel(x, edge_index, edge_vals, edge_index2, edge_vals2,
           W1, b1, W2, b2, W3, b3, W4, b4, Wl, bl):
    x = np.asarray(x, np.float32).astype(np.float16)
    degs = [np.bincount(np.asarray(ei[1]).astype(np.int64), minlength=N)
            for ei in (edge_index, edge_index2)]
    row_of = balance_rows(degs)
    plans = [TowerPlan(edge_index, edge_vals, row_of),
             TowerPlan(edge_index2, edge_vals2, row_of)]

    # ---- launch A: xw = x @ [W1|W3]  (fp8 table out)
    w13 = np.concatenate([np.asarray(W1, np.float32),
                          np.asarray(W3, np.float32)], axis=1).astype(np.float16)
    nc = _make_nc()
    build_A(nc)
    in_maps = []
    for c in range(NCORES):
        xT = np.zeros((NFEAT, NLOCP), np.float16)
        rows = row_of[c * NLOC:(c + 1) * NLOC]
        xT[:, rows] = x[c * NLOC:(c + 1) * NLOC].T
        in_maps.append({"xT": xT, "w13": w13})
    res = _run(nc, in_maps)
    xw = np.zeros((N, 2 * NHID), F8NP)
    for c in range(NCORES):
        rows = row_of[c * NLOC:(c + 1) * NLOC]
        xw[c * NLOC:(c + 1) * NLOC, 0:NHID] = \
            np.asarray(res.results[c]["out0"]).T[rows]
        xw[c * NLOC:(c + 1) * NLOC, NHID:2 * NHID] = \
            np.asarray(res.results[c]["out1"]).T[rows]

    # ---- launch B: h = relu(spmm(xw) + b); hw2 = h @ [W2|W4]
    w24 = np.concatenate([np.asarray(W2, np.float32),
                          np.asarray(W4, np.float32)], axis=1).astype(np.float16)
    b13 = np.stack([np.asarray(b1, np.float32),
                    np.asarray(b3, np.float32)], axis=1)
    nc = _make_nc()
    build_B(nc, plans)
    in_maps = []
    for c in range(NCORES):
        m = {"w24": w24, "b13": b13}
        for tw in (0, 1):
            m[f"msgs{tw}"] = plans[tw].gather_core(
                xw[:, tw * NHID:(tw + 1) * NHID], c)
            m[f"sel{tw}"] = plans[tw].sel[c]
            m[f"dl{tw}"] = plans[tw].dl[c]
            m[f"vl{tw}"] = plans[tw].vl[c]
        in_maps.append(m)
    res = _run(nc, in_maps)
    hw2 = np.zeros((N, 2 * NCLASS), np.float16)
    for c in range(NCORES):
        full = np.asarray(res.results[c]["out"]).transpose(1, 0, 2)
        rows = row_of[c * NLOC:(c + 1) * NLOC]
        hw2[c * NLOC:(c + 1) * NLOC] = full.reshape(NLOCP, 2 * NCLASS)[rows]

    # ---- launch C: o = spmm(hw2) + b; gated fusion; log_softmax
    wl_f = np.asarray(Wl, np.float32)
    wl = np.concatenate([wl_f[0:NCLASS], wl_f[NCLASS:2 * NCLASS]],
                        axis=1).astype(np.float16)
    bias = np.stack([np.asarray(b2, np.float32),
                     np.asarray(b4, np.float32),
                     -np.asarray(bl, np.float32)], axis=1)
    nc = _make_nc()
    build_C(nc, plans)
    in_maps = []
    for c in range(NCORES):
        m = {"wl": wl, "bias": bias}
        for tw in (0, 1):
            m[f"msgs{tw}"] = plans[tw].gather_core(
                hw2[:, tw * NCLASS:(tw + 1) * NCLASS], c)
            m[f"sel{tw}"] = plans[tw].sel[c]
            m[f"dl{tw}"] = plans[tw].dl[c]
            m[f"vl{tw}"] = plans[tw].vl[c]
        in_maps.append(m)
    res = _run(nc, in_maps)
    out = np.zeros((N, NCLASS), np.float32)
    for c in range(NCORES):
        full = np.asarray(res.results[c]["out"]).transpose(1, 0, 2)
        rows = row_of[c * NLOC:(c + 1) * NLOC]
        out[c * NLOC:(c + 1) * NLOC] = full.reshape(NLOCP, NCLASS)[rows]
    return out


# revision 74
# speedup vs baseline: 1.0231x; 1.0111x over previous
"""Trainium2 Bass kernel for the two-tower GCN (nn_GCN2).

Distribution: nodes partitioned by destination range across 8 cores
(graph parallel). All floating-point math runs on device across 3 SPMD
launches; the host only does index manipulation (edge sorting, row
gathering by static indices, dtype casts of inputs) and the inter-launch
reshard/halo-exchange, exactly like the sharding contract allows:

  A: xw  = x @ [W1|W3]                 (node-sharded dense matmul, fp8 out)
  B: h   = relu(spmm(A, xw) + b); hw2 = h @ [W2|W4]    (per dst window)
  C: o   = spmm(A, hw2) + b; gated fusion; log_softmax (per dst window)

The irregular gather of source features is resolved on the host between
launches: since the edge list is static, the per-edge message stream
msgs[chunk, slot, :] = table[src[chunk, slot]] is a pure row-gather of
the previous launch's output, staged partition-major so the device
streams it at full contiguous-DMA bandwidth. The segment-sum runs on the
tensor engine: edges are sorted by destination, so each chunk of 128
edges lands in one 32-wide destination window and
psum[:, win] += msgs_chunk^T @ sel_chunk with a host-built fp8 selector
sel[slot, d] = val * (dst_local == d).
"""
from contextlib import ExitStack

import numpy as np

import concourse.bass as bass
import concourse.tile as tile
from concourse import bacc, mybir
from concourse.bass_utils import run_bass_kernel_spmd
from concourse.masks import make_identity

P = 128
NCORES = 8
N = 50000
E = 800000
NFEAT = 512
NHID = 128
NCLASS = 40
NLOC = N // NCORES             # 6250 real nodes per core
NT128 = 50                     # 128-row blocks per core
NLOCP = NT128 * P              # 6400 padded rows per core (slack for packing)
W = 32                         # dst window width (sel columns)
NWIN = NLOCP // W              # 200 windows per core
SUP = 512                      # dsts per PSUM supertile
NSUP = (NLOCP + SUP - 1) // SUP  # 13 (last one is 256 wide)

f16 = mybir.dt.float16
f32 = mybir.dt.float32
f8 = mybir.dt.float8e4
ACT = mybir.ActivationFunctionType
ALU = mybir.AluOpType
F8NP = mybir.dt.np(f8)


def _cdiv(a, b):
    return (a + b - 1) // b


# ---------------------------------------------------------------- host prep

def balance_rows(degs):
    """Assign each core's nodes to 32-slot windows so that both towers'
    per-window edge counts stay <= 4*128 (pure index manipulation).

    Returns row_of: [N] padded row index of each node within its core.
    """
    deg0, deg1 = degs
    row_of = np.empty(N, np.int64)
    for c in range(NCORES):
        nodes = np.arange(c * NLOC, (c + 1) * NLOC)
        nodes = nodes[np.argsort(-(deg0[nodes] + deg1[nodes]), kind="stable")]
        s0 = np.zeros(NWIN)
        s1 = np.zeros(NWIN)
        cnt = np.zeros(NWIN, np.int64)
        for nd in nodes:
            m = np.where(cnt < 32,
                         np.maximum(s0 + deg0[nd], s1 + deg1[nd]), 1e18)
            w = int(np.argmin(m))
            row_of[nd] = w * W + cnt[w]
            s0[w] += deg0[nd]
            s1[w] += deg1[nd]
            cnt[w] += 1
    return row_of


class TowerPlan:
    """Edge preprocessing for one tower (one graph).

    Sorts each core's in-edges by destination window (32-wide), splits
    them into chunks of 128 slots, pads every (window) to the max chunk
    count over cores so all cores run one program, and records per-slot
    (src, dst_col, val).

    Produces:
      nch          : padded chunk count (same for all cores)
      cs           : [NWIN+1] chunk range per window
      srcs         : [NCORES, nch*128] int32 source row, -1 for pads
      sel          : [NCORES, 128, nch, W] f16 selector (val at dst col)
    """

    def __init__(self, edge_index, edge_vals, row_of):
        src = np.asarray(edge_index[0]).astype(np.int64)
        dst = np.asarray(edge_index[1]).astype(np.int64)
        vals = np.asarray(edge_vals).astype(np.float32)

        core = dst // NLOC
        ldst = row_of[dst]
        win = ldst // W
        col = ldst - win * W

        counts = np.zeros((NCORES, NWIN), np.int64)
        np.add.at(counts, (core, win), 1)
        chunk_cnt = np.maximum(_cdiv(counts, P).max(axis=0), 1)  # [NWIN]
        self.cs = np.concatenate([[0], np.cumsum(chunk_cnt)])
        self.nch = int(self.cs[-1])

        order = np.lexsort((ldst, win, core))
        so_core, so_win = core[order], win[order]
        so_src, so_col, so_val = src[order], col[order], vals[order]
        gkey = so_core * NWIN + so_win
        gstart = np.r_[0, np.flatnonzero(np.diff(gkey)) + 1]
        glen = np.diff(np.r_[gstart, len(gkey)])
        rank = np.arange(len(gkey)) - np.repeat(gstart, glen)
        slot = (self.cs[so_win] * P + rank).astype(np.int64)

        nslot = self.nch * P
        self.srcs = np.full((NCORES, nslot), -1, np.int32)
        cola = np.zeros((NCORES, nslot), np.int64)
        vala = np.zeros((NCORES, nslot), np.float32)
        flat = so_core * nslot + slot
        self.srcs.reshape(-1)[flat] = so_src.astype(np.int32)
        cola.reshape(-1)[flat] = so_col
        vala.reshape(-1)[flat] = so_val

        # compact per-slot forms for on-device sel builds
        self.dl = np.ascontiguousarray(
            cola.reshape(NCORES, self.nch, P).transpose(0, 2, 1)
        ).astype(np.float32)
        self.vl = np.ascontiguousarray(
            vala.reshape(NCORES, self.nch, P).transpose(0, 2, 1)
        ).astype(np.float32)

        # sel[c, p, ch, col] = val of slot (ch, p) if its dst col matches
        sel = np.zeros((NCORES, self.nch, P, W), np.float16)
        ci = np.arange(nslot) // P
        pi = np.arange(nslot) % P
        v8 = vala.astype(np.float16)
        for c in range(NCORES):
            sel[c, ci, pi, cola[c]] = v8[c]
            # pads: col 0 with val 0 already zero
        self.sel = np.ascontiguousarray(sel.transpose(0, 2, 1, 3))

    def gather_core(self, table, c):
        """msgs = table[srcs[c]] staged [128, nch, F]; pads -> 0."""
        tab = np.concatenate(
            [np.zeros((1, table.shape[1]), table.dtype), table], axis=0
        )
        m = tab[self.srcs[c] + 1]                          # [nch*128, F]
        m = m.reshape(self.nch, P, -1).transpose(1, 0, 2)
        return np.ascontiguousarray(m)


# ---------------------------------------------------------------- kernels

def build_A(nc):
    """xw^T = [W1|W3]^T @ x^T, written as two feature-half tables
    out0/out1 [128, NLOCP] fp8 (host transposes for the row gather)."""
    xT = nc.dram_tensor("xT", [NFEAT, NLOCP], f16, kind="ExternalInput").ap()
    w13 = nc.dram_tensor("w13", [NFEAT, 2 * NHID], f16, kind="ExternalInput").ap()
    out0 = nc.dram_tensor("out0", [P, NLOCP], f8, kind="ExternalOutput").ap()
    out1 = nc.dram_tensor("out1", [P, NLOCP], f8, kind="ExternalOutput").ap()
    outs = [out0, out1]
    KCH = NFEAT // P              # 4
    STRIP = 3 * SUP               # 1536 cols per xT strip (SUP-aligned)
    NS = _cdiv(NLOCP, STRIP)      # 5 (last strip 256)

    with tile.TileContext(nc) as tc, ExitStack() as ctx:
        big = ctx.enter_context(tc.tile_pool(name="big", bufs=1))
        ob_pool = ctx.enter_context(tc.tile_pool(name="obp", bufs=4))
        psum = ctx.enter_context(tc.tile_pool(name="ps", bufs=6, space="PSUM"))

        w_t = big.tile([P, KCH, 2 * NHID], f16, tag="w")
        nc.sync.dma_start(
            w_t[:], w13.rearrange("(k p) f -> p k f", p=P)[:]
        )
        xts = []
        for s in range(NS):
            c0 = s * STRIP
            cw = min(STRIP, NLOCP - c0)
            per_k = []
            for k in range(KCH):
                t = big.tile([P, STRIP], f16, tag=f"xts{s}_{k}")
                nc.sync.dma_start(
                    t[:, :cw], xT[k * P:(k + 1) * P, c0:c0 + cw]
                )
                per_k.append(t)
            xts.append(per_k)

        for s in range(NS):
            c0 = s * STRIP
            cw = min(STRIP, NLOCP - c0)
            ob0 = ob_pool.tile([P, STRIP], f8, tag="ob0")
            ob1 = ob_pool.tile([P, STRIP], f8, tag="ob1")
            obs = [ob0, ob1]
            for sup in range(_cdiv(cw, SUP)):
                lc = sup * SUP
                ncols = min(SUP, cw - lc)
                for half in (0, 1):
                    ps = psum.tile([P, SUP], f32, tag="ps")
                    for k in range(KCH):
                        nc.tensor.matmul(
                            ps[:, :ncols],
                            lhsT=w_t[:, k, half * P:(half + 1) * P],
                            rhs=xts[s][k][:, lc:lc + ncols],
                            start=(k == 0), stop=(k == KCH - 1),
                        )
                    nc.vector.tensor_copy(
                        obs[half][:, lc:lc + ncols], ps[:, :ncols]
                    )
            for half in (0, 1):
                nc.sync.dma_start(outs[half][:, c0:c0 + cw], obs[half][:, :cw])
    nc.compile()
    return nc


def _sup_windows(sup):
    w0 = sup * (SUP // W)
    w1 = min(NWIN, w0 + SUP // W)
    return w0, w1


def _spmm_supertile(nc, plan, msgs_t, sel_t, ps, sup, g0):
    """Accumulate all chunks of supertile `sup` into psum tile ps."""
    w0, w1 = _sup_windows(sup)
    for w in range(w0, w1):
        lo, hi = int(plan.cs[w]), int(plan.cs[w + 1])
        for k, ch in enumerate(range(lo, hi)):
            nc.tensor.matmul(
                ps[:, (w - w0) * W:(w - w0 + 1) * W],
                lhsT=msgs_t[:, ch - g0, :],
                rhs=sel_t[:, ch - g0, :],
                start=(k == 0), stop=(k == hi - lo - 1),
            )


BUILD_SEL_B = 6      # tower-1 supertiles < this build sel on DVE


def build_B(nc, plans):
    nch = [p.nch for p in plans]
    msgs_d = [
        nc.dram_tensor(f"msgs{tw}", [P, nch[tw], NHID], f8,
                       kind="ExternalInput").ap()
        for tw in (0, 1)
    ]
    sel_d = [
        nc.dram_tensor(f"sel{tw}", [P, nch[tw], W], f16,
                       kind="ExternalInput").ap()
        for tw in (0, 1)
    ]
    dl_d = [
        nc.dram_tensor(f"dl{tw}", [P, nch[tw]], f32, kind="ExternalInput").ap()
        for tw in (0, 1)
    ]
    vl_d = [
        nc.dram_tensor(f"vl{tw}", [P, nch[tw]], f32, kind="ExternalInput").ap()
        for tw in (0, 1)
    ]
    w24 = nc.dram_tensor("w24", [NHID, 2 * NCLASS], f16, kind="ExternalInput").ap()
    b13 = nc.dram_tensor("b13", [NHID, 2], f32, kind="ExternalInput").ap()
    out = nc.dram_tensor("out", [P, NT128, 2 * NCLASS], f16,
                         kind="ExternalOutput").ap()

    with tile.TileContext(nc) as tc, ExitStack() as ctx:
        consts = ctx.enter_context(tc.tile_pool(name="consts", bufs=1))
        w24_t = consts.tile([NHID, 2 * NCLASS], f16)
        nc.sync.dma_start(w24_t[:], w24[:])
        b13_t = consts.tile([NHID, 2], f32)
        nc.sync.dma_start(b13_t[:], b13[:])
        ob = consts.tile([P, NT128, 2 * NCLASS], f16, tag="ob")
        it32 = consts.tile([P, W], mybir.dt.int32, tag="it32")
        nc.gpsimd.iota(it32[:], pattern=[[1, W]], base=0, channel_multiplier=0)
        iota = consts.tile([P, W], f16, tag="iota")
        nc.vector.tensor_copy(iota[:], it32[:])
        dl_t, vl_t = {}, {}
        for tw in (0, 1):
            t = consts.tile([P, nch[tw]], f32, tag=f"dl{tw}")
            nc.sync.dma_start(t[:], dl_d[tw][:])
            dl_t[tw] = t
            t = consts.tile([P, nch[tw]], f32, tag=f"vl{tw}")
            nc.sync.dma_start(t[:], vl_d[tw][:])
            vl_t[tw] = t

        gmax = max(
            int(p.cs[_sup_windows(s)[1]] - p.cs[_sup_windows(s)[0]])
            for p in plans for s in range(NSUP)
        )
        mpool = ctx.enter_context(tc.tile_pool(name="msgs", bufs=6))
        spool = ctx.enter_context(tc.tile_pool(name="sel", bufs=7))
        hpool = ctx.enter_context(tc.tile_pool(name="h", bufs=3))
        psum = ctx.enter_context(tc.tile_pool(name="ps", bufs=3, space="PSUM"))
        psum2 = ctx.enter_context(tc.tile_pool(name="ps2", bufs=3, space="PSUM"))

        units = []
        for tw in (0, 1):
            plan = plans[tw]
            for sup in range(NSUP):
                w0, w1 = _sup_windows(sup)
                units.append((tw, sup, int(plan.cs[w0]), int(plan.cs[w1]),
                              (w1 - w0) * W))

        def sel_prep(i):
            tw, sup, g0, g1, ncols = units[i]
            sel_t = spool.tile([P, gmax, W], f16, tag="s")
            if tw == 1 and sup in (0, 1, 7, 8, 10):
                eng = nc.gpsimd          # Pool engine is otherwise idle
            elif tw == 0 or sup < BUILD_SEL_B + 1 or sup == 9:
                eng = nc.vector
            else:
                nc.sync.dma_start(
                    sel_t[:, :g1 - g0, :], sel_d[tw][:, g0:g1, :]
                )
                return sel_t
            for ch in range(g0, g1):
                eng.tensor_scalar(
                    out=sel_t[:, ch - g0, :], in0=iota[:],
                    scalar1=dl_t[tw][:, ch:ch + 1],
                    scalar2=vl_t[tw][:, ch:ch + 1],
                    op0=ALU.is_equal, op1=ALU.mult,
                )
            return sel_t

        sel_ready = [sel_prep(0), sel_prep(1), sel_prep(2)]
        for i, (tw, sup, g0, g1, ncols) in enumerate(units):
            plan = plans[tw]
            msgs_t = mpool.tile([P, gmax, NHID], f8, tag="m")
            nc.sync.dma_start(msgs_t[:, :g1 - g0, :], msgs_d[tw][:, g0:g1, :])
            sel_t = sel_ready.pop(0)
            if i + 3 < len(units):
                sel_ready.append(sel_prep(i + 3))

            ps = psum.tile([NHID, SUP], f32, tag="ps")
            _spmm_supertile(nc, plan, msgs_t, sel_t, ps, sup, g0)

            # h = relu(ps + b) in f16, then h @ W2 per 128-dst slice
            hT = hpool.tile([NHID, SUP], f16, tag="hT")
            nc.scalar.activation(
                out=hT[:, :ncols], in_=ps[:, :ncols], func=ACT.Relu,
                bias=b13_t[:, tw:tw + 1], scale=1.0,
            )
            for j in range(ncols // P):
                ps2 = psum2.tile([P, NCLASS], f32, tag="ps2")
                nc.tensor.matmul(
                    ps2[:], lhsT=hT[:, j * P:(j + 1) * P],
                    rhs=w24_t[:, tw * NCLASS:(tw + 1) * NCLASS],
                    start=True, stop=True,
                )
                t128 = sup * (SUP // P) + j
                nc.scalar.copy(
                    ob[:, t128, tw * NCLASS:(tw + 1) * NCLASS], ps2[:]
                )
            if tw == 1 and sup == 5:
                # blocks 0..23 are complete for both towers: stream them
                # out now so only a small output write remains at the end
                nc.sync.dma_start(out[:, 0:24, :], ob[:, 0:24, :])
            if tw == 1 and sup == 9:
                nc.sync.dma_start(out[:, 24:40, :], ob[:, 24:40, :])
            if tw == 1 and sup == 11:
                nc.sync.dma_start(out[:, 40:48, :], ob[:, 40:48, :])
        nc.sync.dma_start(out[:, 48:NT128, :], ob[:, 48:NT128, :])
    nc.compile()
    return nc


def build_C(nc, plans):
    nch = [p.nch for p in plans]
    msgs_d = [
        nc.dram_tensor(f"msgs{tw}", [P, nch[tw], NCLASS], f16,
                       kind="ExternalInput").ap()
        for tw in (0, 1)
    ]
    sel_d = [
        nc.dram_tensor(f"sel{tw}", [P, nch[tw], W], f16,
                       kind="ExternalInput").ap()
        for tw in (0, 1)
    ]
    wl = nc.dram_tensor("wl", [NCLASS, 2 * NCLASS], f16, kind="ExternalInput").ap()
    bias = nc.dram_tensor("bias", [NCLASS, 3], f32, kind="ExternalInput").ap()
    dl_d = [
        nc.dram_tensor(f"dl{tw}", [P, nch[tw]], f32, kind="ExternalInput").ap()
        for tw in (0, 1)
    ]
    vl_d = [
        nc.dram_tensor(f"vl{tw}", [P, nch[tw]], f32, kind="ExternalInput").ap()
        for tw in (0, 1)
    ]
    out = nc.dram_tensor("out", [P, NT128, NCLASS], f32,
                         kind="ExternalOutput").ap()

    with tile.TileContext(nc) as tc, ExitStack() as ctx:
        consts = ctx.enter_context(tc.tile_pool(name="consts", bufs=1))
        wl_t = consts.tile([NCLASS, 2 * NCLASS], f16)
        nc.sync.dma_start(wl_t[:], wl[:])
        bias_t = consts.tile([NCLASS, 3], f32)   # cols: b2, b4, bl
        nc.sync.dma_start(bias_t[:], bias[:])
        it32 = consts.tile([P, W], mybir.dt.int32, tag="it32")
        nc.gpsimd.iota(it32[:], pattern=[[1, W]], base=0, channel_multiplier=0)
        iota = consts.tile([P, W], f16, tag="iota")
        nc.vector.tensor_copy(iota[:], it32[:])
        dl_t, vl_t = {}, {}
        for tw in (0, 1):
            t = consts.tile([P, nch[tw]], f32, tag=f"dl{tw}")
            nc.sync.dma_start(t[:], dl_d[tw][:])
            dl_t[tw] = t
            t = consts.tile([P, nch[tw]], f32, tag=f"vl{tw}")
            nc.sync.dma_start(t[:], vl_d[tw][:])
            vl_t[tw] = t
        identf = consts.tile([NCLASS, NCLASS], f16, tag="identf")
        ident32 = consts.tile([P, P], f32, tag="ident32")
        make_identity(nc, ident32[:])
        nc.vector.tensor_copy(identf[:], ident32[0:NCLASS, 0:NCLASS])
        ob = consts.tile([P, NT128, NCLASS], f32, tag="ob")
        oT0 = consts.tile([NCLASS, NLOCP], f16, tag="oT0")
        oT1 = consts.tile([NCLASS, NLOCP], f16, tag="oT1")
        oT = [oT0, oT1]
        t_all = consts.tile([P, NT128, NCLASS], f16, tag="t_all")
        negmax_all = consts.tile([P, NT128], f32, tag="negmax")
        esum_all = consts.tile([P, NT128], f32, tag="esum")
        lse_all = consts.tile([P, NT128], f32, tag="lse")

        mpool = ctx.enter_context(tc.tile_pool(name="msgs", bufs=6))
        spool = ctx.enter_context(tc.tile_pool(name="sel", bufs=6))
        work = ctx.enter_context(tc.tile_pool(name="work", bufs=6))
        psum = ctx.enter_context(tc.tile_pool(name="ps", bufs=3, space="PSUM"))
        psumg = ctx.enter_context(tc.tile_pool(name="psg", bufs=2, space="PSUM"))
        psum2 = ctx.enter_context(tc.tile_pool(name="ps2", bufs=2, space="PSUM"))

        sup_outs = []

        def emit_softmax(sup, outT, ncols, c0):
            nblk = ncols // P
            t0 = sup * (SUP // P)
            ps_t4 = psum2.tile([P, 4, NCLASS], f16, tag="pst")
            for j in range(nblk):
                nc.tensor.transpose(
                    out=ps_t4[:, j, :], in_=outT[:, j * P:(j + 1) * P],
                    identity=identf[:],
                )
            nc.vector.tensor_reduce(
                out=negmax_all[:, t0:t0 + nblk], in_=ps_t4[:, :nblk, :],
                axis=mybir.AxisListType.X, op=ALU.max, negate=True,
            )
            for j in range(nblk):
                etmp = work.tile([P, NCLASS], f16, tag="etmp")
                nc.scalar.activation(
                    out=etmp[:], in_=ps_t4[:, j, :], func=ACT.Exp,
                    bias=negmax_all[:, t0 + j:t0 + j + 1], scale=1.0,
                    accum_out=esum_all[:, t0 + j:t0 + j + 1],
                )
            nc.vector.tensor_copy(t_all[:, t0:t0 + nblk, :], ps_t4[:, :nblk, :])

        # ---- per supertile: spmm both towers, then fused gate+softmax
        gmax = max(
            int(p.cs[_sup_windows(s)[1]] - p.cs[_sup_windows(s)[0]])
            for p in plans for s in range(NSUP)
        )
        units = []
        for sup in range(NSUP):
            w0, w1 = _sup_windows(sup)
            for tw in (0, 1):
                plan = plans[tw]
                units.append((tw, sup, int(plan.cs[w0]), int(plan.cs[w1])))

        def sel_prep(i):
            tw, sup, g0, g1 = units[i]
            sel_t = spool.tile([P, gmax, W], f16, tag="s")
            if tw == 1 and sup in (0, 1):
                eng = nc.gpsimd          # Pool engine is otherwise idle
            elif tw == 0 and sup < 7:
                eng = nc.vector
            else:
                nc.sync.dma_start(
                    sel_t[:, :g1 - g0, :], sel_d[tw][:, g0:g1, :]
                )
                return sel_t
            for ch in range(g0, g1):
                eng.tensor_scalar(
                    out=sel_t[:, ch - g0, :], in0=iota[:],
                    scalar1=dl_t[tw][:, ch:ch + 1],
                    scalar2=vl_t[tw][:, ch:ch + 1],
                    op0=ALU.is_equal, op1=ALU.mult,
                )
            return sel_t

        sel_ready = [sel_prep(0), sel_prep(1)]
        for sup in range(NSUP):
            w0, w1 = _sup_windows(sup)
            ncols = (w1 - w0) * W
            c0 = sup * SUP
            for tw in (0, 1):
                i = sup * 2 + tw
                plan = plans[tw]
                g0, g1 = int(plan.cs[w0]), int(plan.cs[w1])
                msgs_t = mpool.tile([P, gmax, NCLASS], f16, tag="m")
                nc.sync.dma_start(msgs_t[:, :g1 - g0, :], msgs_d[tw][:, g0:g1, :])
                sel_t = sel_ready.pop(0)
                if i + 2 < len(units):
                    sel_ready.append(sel_prep(i + 2))

                ps = psum.tile([NCLASS, SUP], f32, tag="ps")
                _spmm_supertile(nc, plan, msgs_t, sel_t, ps, sup, g0)
                if tw == 0:
                    nc.scalar.activation(
                        out=oT[tw][:, c0:c0 + ncols], in_=ps[:, :ncols],
                        func=ACT.Identity, bias=bias_t[:, tw:tw + 1], scale=1.0,
                    )
                else:
                    nc.vector.tensor_scalar(
                        out=oT[tw][:, c0:c0 + ncols],
                        in0=ps[:, :ncols],
                        scalar1=bias_t[:, tw:tw + 1], scalar2=None, op0=ALU.add,
                    )
            ps_g = psumg.tile([NCLASS, SUP], f32, tag="psg")
            nc.tensor.matmul(
                ps_g[:, :ncols], lhsT=wl_t[:, 0:NCLASS],
                rhs=oT[0][:, c0:c0 + ncols],
                start=True, stop=False,
            )
            nc.tensor.matmul(
                ps_g[:, :ncols], lhsT=wl_t[:, NCLASS:2 * NCLASS],
                rhs=oT[1][:, c0:c0 + ncols],
                start=False, stop=True,
            )
            if sup == NSUP - 1:
                # early finish for blocks of sups 0..10: their esums are
                # ready; the Ln table reload and the 44 combines hide
                # under this last supertile's stream
                nfin = (NSUP - 2) * (SUP // P)
                nc.scalar.activation(
                    out=lse_all[:, 0:nfin], in_=esum_all[:, 0:nfin],
                    func=ACT.Ln,
                )
                for t128 in range(nfin):
                    nc.vector.tensor_scalar(
                        out=ob[:, t128, :], in0=t_all[:, t128, :],
                        scalar1=negmax_all[:, t128:t128 + 1],
                        scalar2=lse_all[:, t128:t128 + 1],
                        op0=ALU.add, op1=ALU.subtract,
                    )
                nc.sync.dma_start(out[:, 0:nfin, :], ob[:, 0:nfin, :])

            # gate = 1 / (1 + exp(-(z + bl))); bias col 2 holds -bl
            eneg = work.tile([NCLASS, SUP], f32, tag="eneg")
            nc.scalar.activation(
                out=eneg[:, :ncols], in_=ps_g[:, :ncols], func=ACT.Exp,
                bias=bias_t[:, 2:3], scale=-1.0,
            )
            # softmax of the previous supertile: its inputs are ready, and
            # emitting it here keeps DVE busy while ACT computes eneg
            if len(sup_outs) > 0:
                emit_softmax(*sup_outs.pop(0))
            dif = work.tile([NCLASS, SUP], f16, tag="dif")
            nc.vector.tensor_tensor(
                out=dif[:, :ncols], in0=oT[0][:, c0:c0 + ncols],
                in1=oT[1][:, c0:c0 + ncols], op=ALU.subtract,
            )
            den = work.tile([NCLASS, SUP], f32, tag="den")
            nc.vector.tensor_scalar(
                out=den[:, :ncols], in0=eneg[:, :ncols], scalar1=1.0,
                scalar2=None, op0=ALU.add,
            )
            gt = work.tile([NCLASS, SUP], f32, tag="gt")
            nc.vector.reciprocal(gt[:, :ncols], den[:, :ncols])
            nc.vector.tensor_tensor(out=dif[:, :ncols], in0=gt[:, :ncols],
                                    in1=dif[:, :ncols], op=ALU.mult)
            outT = work.tile([NCLASS, SUP], f16, tag="outT")
            nc.vector.tensor_tensor(
                out=outT[:, :ncols], in0=oT[1][:, c0:c0 + ncols],
                in1=dif[:, :ncols], op=ALU.add,
            )
            sup_outs.append((sup, outT, ncols, c0))
        emit_softmax(*sup_outs.pop(0))

        nfin = (NSUP - 2) * (SUP // P)
        nc.scalar.activation(out=lse_all[:, nfin:NT128],
                             in_=esum_all[:, nfin:NT128], func=ACT.Ln)
        for t128 in range(nfin, NT128):
            nc.vector.tensor_scalar(
                out=ob[:, t128, :], in0=t_all[:, t128, :],
                scalar1=negmax_all[:, t128:t128 + 1],
                scalar2=lse_all[:, t128:t128 + 1],
                op0=ALU.add, op1=ALU.subtract,
            )
        nc.sync.dma_start(out[:, nfin:NT128, :], ob[:, nfin:NT128, :])
    nc.compile()
    return nc


# ---------------------------------------------------------------- driver

TRACE = False          # set by test.py to collect per-launch artifacts
LAST_NCS = []          # built Bass modules per launch when TRACE


def _run(nc, in_maps):
    if TRACE:
        LAST_NCS.append(nc)
    return run_bass_kernel_spmd(nc, in_maps, core_ids=list(range(NCORES)))


def _make_nc():
    return bacc.Bacc(
        "TRN2", target_bir_lowering=False, debug=False,
        num_devices=NCORES, num_swdge_queues=1,
    )


def kernel(x, edge_index, edge_vals, edge_index2, edge_vals2,
           W1, b1, W2, b2, W3, b3, W4, b4, Wl, bl):
    x = np.asarray(x, np.float32).astype(np.float16)
    degs = [np.bincount(np.asarray(ei[1]).astype(np.int64), minlength=N)
            for ei in (edge_index, edge_index2)]
    row_of = balance_rows(degs)
    plans = [TowerPlan(edge_index, edge_vals, row_of),
             TowerPlan(edge_index2, edge_vals2, row_of)]

    # ---- launch A: xw = x @ [W1|W3]  (fp8 table out)
    w13 = np.concatenate([np.asarray(W1, np.float32),
                          np.asarray(W3, np.float32)], axis=1).astype(np.float16)
    nc = _make_nc()
    build_A(nc)
    in_maps = []
    for c in range(NCORES):
        xT = np.zeros((NFEAT, NLOCP), np.float16)
        rows = row_of[c * NLOC:(c + 1) * NLOC]
        xT[:, rows] = x[c * NLOC:(c + 1) * NLOC].T
        in_maps.append({"xT": xT, "w13": w13})
    res = _run(nc, in_maps)
    xw = np.zeros((N, 2 * NHID), F8NP)
    for c in range(NCORES):
        rows = row_of[c * NLOC:(c + 1) * NLOC]
        xw[c * NLOC:(c + 1) * NLOC, 0:NHID] = \
            np.asarray(res.results[c]["out0"]).T[rows]
        xw[c * NLOC:(c + 1) * NLOC, NHID:2 * NHID] = \
            np.asarray(res.results[c]["out1"]).T[rows]

    # ---- launch B: h = relu(spmm(xw) + b); hw2 = h @ [W2|W4]
    w24 = np.concatenate([np.asarray(W2, np.float32),
                          np.asarray(W4, np.float32)], axis=1).astype(np.float16)
    b13 = np.stack([np.asarray(b1, np.float32),
                    np.asarray(b3, np.float32)], axis=1)
    nc = _make_nc()
    build_B(nc, plans)
    in_maps = []
    for c in range(NCORES):
        m = {"w24": w24, "b13": b13}
        for tw in (0, 1):
            m[f"msgs{tw}"] = plans[tw].gather_core(
                xw[:, tw * NHID:(tw + 1) * NHID], c)
            m[f"sel{tw}"] = plans[tw].sel[c]
            m[f"dl{tw}"] = plans[tw].dl[c]
            m[f"vl{tw}"] = plans[tw].vl[c]
        in_maps.append(m)
    res = _run(nc, in_maps)
    hw2 = np.zeros((N, 2 * NCLASS), np.float16)
    for c in range(NCORES):
        full = np.asarray(res.results[c]["out"]).transpose(1, 0, 2)
        rows = row_of[c * NLOC:(c + 1) * NLOC]
        hw2[c * NLOC:(c + 1) * NLOC] = full.reshape(NLOCP, 2 * NCLASS)[rows]

    # ---- launch C: o = spmm(hw2) + b; gated fusion; log_softmax
    wl_f = np.asarray(Wl, np.float32)
    wl = np.concatenate([wl_f[0:NCLASS], wl_f[NCLASS:2 * NCLASS]],
                        axis=1).astype(np.float16)
    bias = np.stack([np.asarray(b2, np.float32),
                     np.asarray(b4, np.float32),
                     -np.asarray(bl, np.float32)], axis=1)
    nc = _make_nc()
    build_C(nc, plans)
    in_maps = []
    for c in range(NCORES):
        m = {"wl": wl, "bias": bias}
        for tw in (0, 1):
            m[f"msgs{tw}"] = plans[tw].gather_core(
                hw2[:, tw * NCLASS:(tw + 1) * NCLASS], c)
            m[f"sel{tw}"] = plans[tw].sel[c]
            m[f"dl{tw}"] = plans[tw].dl[c]
            m[f"vl{tw}"] = plans[tw].vl[c]
        in_maps.append(m)
    res = _run(nc, in_maps)
    out = np.zeros((N, NCLASS), np.float32)
    for c in range(NCORES):
        full = np.asarray(res.results[c]["out"]).transpose(1, 0, 2)
        rows = row_of[c * NLOC:(c + 1) * NLOC]
        out[c * NLOC:(c + 1) * NLOC] = full.reshape(NLOCP, NCLASS)[rows]
    return out
